# revision 13
# baseline (speedup 1.0000x reference)
"""BatchNorm over batch axis (N=131072, D=512) on 8 trn2 NeuronCores.

Strategy (per sharding hint): shard X row-wise across 8 cores. Each core
computes per-feature partial sums (sum x, sum x^2) over its 16384 rows,
all-reduces the two D-length vectors across cores, derives per-feature
scale = gamma * rsqrt(var) and bias = beta - mean * scale, then streams
its shard again applying Y = X * scale + bias.

Memory-bound: per core 2 reads + 1 write of 33.5 MB => ~100 MB @ ~358 GB/s.

Engine budget per 2 MiB macro-tile (DMA 5.9 us):
  pass 1: DVE acc+=x (~5.3 us), ACT square (~3.4 us), PE 8 f32r
          ones-matmuls accumulating sum(x^2) into PSUM (~3.5 us).
  pass 2: DVE mult (~5.3 us), GpSimd add (~9 us) vs 11.7 us r+w DMA.
X loads ride the sync queue exclusively; Y stores and stats DMAs ride the
scalar queue, so load triggers are never blocked behind a semaphore wait
and prefetch runs 9 tiles deep through the all-reduce bubble.
"""

import numpy as np
from contextlib import ExitStack

import concourse.bass as bass
import concourse.bacc as bacc
import concourse.tile as tile
from concourse import mybir
from concourse.bass_utils import run_bass_kernel_spmd

N, D = 131072, 512
NCORES = 8
NP = N // NCORES  # rows per core
P = 128           # SBUF partitions
RB = 8            # 128-row blocks per macro tile -> 1024 rows, 2 MiB per DMA
F32 = mybir.dt.float32
F32R = mybir.dt.float32r

_cache = {}


def _build(np_rows=NP, n_total=N):
    rows_per_tile = P * RB
    nt = np_rows // rows_per_tile
    assert nt * rows_per_tile == np_rows

    nc = bacc.Bacc(num_devices=NCORES)
    X = nc.declare_dram_parameter("X", [np_rows, D], F32, isOutput=False)
    gamma = nc.declare_dram_parameter("gamma", [1, D], F32, isOutput=False)
    beta = nc.declare_dram_parameter("beta", [1, D], F32, isOutput=False)
    Y = nc.declare_dram_parameter("Y", [np_rows, D], F32, isOutput=True)
    cc_in = nc.dram_tensor("cc_in", [1, 2, D], F32)
    cc_out = nc.dram_tensor("cc_out", [1, 2, D], F32, addr_space="Shared")
    cc_inB = nc.dram_tensor("cc_inB", [1, 2, D], F32)
    cc_outB = nc.dram_tensor("cc_outB", [1, 2, D], F32, addr_space="Shared")
    bar_in = nc.dram_tensor("bar_in", [1, 8], F32)
    bar_out = nc.dram_tensor("bar_out", [1, 8], F32, addr_space="Shared")

    Xv = X[:].rearrange("(t p b) d -> t p b d", p=P, b=RB)
    Yv = Y[:].rearrange("(t p b) d -> t p b d", p=P, b=RB)

    with tile.TileContext(nc) as tc, ExitStack() as ctx:
        stream = ctx.enter_context(tc.tile_pool(name="stream", bufs=5))
        sqpool = ctx.enter_context(tc.tile_pool(name="sq", bufs=2))
        accs = ctx.enter_context(tc.tile_pool(name="accs", bufs=1))
        singles = ctx.enter_context(tc.tile_pool(name="singles", bufs=1))
        psum = ctx.enter_context(tc.tile_pool(name="psum", bufs=1, space="PSUM"))

        # group split: all-reduce A covers tiles [0, a_end) and hides under
        # pass-1 streaming; all-reduce B covers the tail and only pays ring
        # latency (cores are already synced by the start barrier + CC_A)
        a_end = max(1, nt - 4)

        scr = singles.tile([P, 4, D], F32)   # early: barrier/ones src; late: stats scratch

        # early rendezvous barrier: absorbs inter-core kernel-start skew
        # while pass-1 streaming runs
        nc.vector.memset(scr[0:1, 0, 0:8], 0.0)
        nc.gpsimd.dma_start(out=bar_in[:], in_=scr[0:1, 0, 0:8])
        nc.gpsimd.collective_compute(
            "AllReduce",
            mybir.AluOpType.add,
            replica_groups=[list(range(NCORES))],
            ins=[bar_in[:].opt()],
            outs=[bar_out[:].opt()],
        )

        nc.vector.memset(scr[:, 1, 0:1], 1.0)
        ones = singles.tile([P, 1], F32R)
        nc.scalar.copy(ones[:], scr[:, 1, 0:1])
        ps_xA = psum.tile([1, D], F32)
        ps_xT = psum.tile([1, D], F32)
        ps_x2A = psum.tile([1, D], F32)
        ps_x2B = psum.tile([1, D], F32)

        # --- pass 1: per-core partial sums ---
        acc = accs.tile([P, RB, D], F32)  # x sums (DVE wide adds, full rate)
        nc.vector.memset(acc[:], 0.0)

        stageA = singles.tile([1, 2, D], F32)
        stageB = singles.tile([1, 2, D], F32)

        for t in range(nt):
            xt = stream.tile([P, RB, D], F32)
            nc.sync.dma_start(out=xt[:], in_=Xv[t])
            nc.vector.tensor_add(acc[:], acc[:], xt[:])
            sq = sqpool.tile([P, RB, D], F32R)
            nc.scalar.square(sq[:], xt[:])
            ps2 = ps_x2A if t < a_end else ps_x2B
            for b in range(RB):
                nc.tensor.matmul(
                    ps2[:],
                    lhsT=ones[:],
                    rhs=sq[:, b, :],
                    start=(t in (0, a_end) and b == 0),
                    stop=(t in (a_end - 1, nt - 1) and b == RB - 1),
                )

            if t == a_end - 1:
                # fold group-A x sums without destroying acc, all-reduce A
                uA = accs.tile([P, 4, D], F32)
                nc.vector.tensor_add(uA[:], acc[:, 0:4, :], acc[:, 4:8, :])
                nc.vector.tensor_add(uA[:, 0:2, :], uA[:, 0:2, :], uA[:, 2:4, :])
                nc.vector.tensor_add(uA[:, 0, :], uA[:, 0, :], uA[:, 1, :])
                colsA = singles.tile([P, D], F32R, tag="cols")
                nc.scalar.copy(colsA[:], uA[:, 0, :])
                nc.tensor.matmul(ps_xA[:], lhsT=ones[:], rhs=colsA[:],
                                 start=True, stop=True)
                nc.scalar.copy(stageA[:, 0, :], ps_xA[:])
                nc.scalar.copy(stageA[:, 1, :], ps_x2A[:])
                nc.gpsimd.dma_start(out=cc_in[:], in_=stageA[:])
                nc.gpsimd.collective_compute(
                    "AllReduce",
                    mybir.AluOpType.add,
                    replica_groups=[list(range(NCORES))],
                    ins=[cc_in[:].opt()],
                    outs=[cc_out[:].opt()],
                )

        # fold TOTAL x sums (in place), derive group-B = total - A, all-reduce B
        nc.vector.tensor_add(acc[:, 0:4, :], acc[:, 0:4, :], acc[:, 4:8, :])
        nc.vector.tensor_add(acc[:, 0:2, :], acc[:, 0:2, :], acc[:, 2:4, :])
        nc.vector.tensor_add(acc[:, 0, :], acc[:, 0, :], acc[:, 1, :])
        colsT = singles.tile([P, D], F32R, tag="cols")
        nc.scalar.copy(colsT[:], acc[:, 0, :])
        nc.tensor.matmul(ps_xT[:], lhsT=ones[:], rhs=colsT[:],
                         start=True, stop=True)
        nc.vector.tensor_sub(stageB[:, 0, :], ps_xT[:], stageA[:, 0, :])
        nc.scalar.copy(stageB[:, 1, :], ps_x2B[:])
        nc.gpsimd.dma_start(out=cc_inB[:], in_=stageB[:])
        nc.gpsimd.collective_compute(
            "AllReduce",
            mybir.AluOpType.add,
            replica_groups=[list(range(NCORES))],
            ins=[cc_inB[:].opt()],
            outs=[cc_outB[:].opt()],
        )

        # --- stats -> scale/bias, replicated on all partitions (scalar queue) ---
        gb = singles.tile([P, 2, D], F32)
        nc.scalar.dma_start(out=gb[:, 0, :], in_=gamma[:].to_broadcast((P, D)))
        nc.scalar.dma_start(out=gb[:, 1, :], in_=beta[:].to_broadcast((P, D)))
        sumsA = singles.tile([P, 2, D], F32)
        sumsB = singles.tile([P, 2, D], F32)
        nc.scalar.dma_start(out=sumsA[:], in_=cc_out[:].to_broadcast((P, 2, D)))
        nc.scalar.dma_start(out=sumsB[:], in_=cc_outB[:].to_broadcast((P, 2, D)))

        mean2 = singles.tile([P, 2, D], F32)  # [:,0]=mean  [:,1]=E[x^2]
        nc.vector.tensor_add(sumsA[:], sumsA[:], sumsB[:])
        nc.scalar.mul(mean2[:], sumsA[:], 1.0 / n_total)
        var, sd, inv, tmp = scr[:, 0, :], scr[:, 1, :], scr[:, 2, :], scr[:, 3, :]
        nc.scalar.square(var, mean2[:, 0, :])
        nc.vector.tensor_sub(var, mean2[:, 1, :], var)
        nc.scalar.sqrt(sd, var)
        nc.vector.reciprocal(inv, sd)

        SB = singles.tile([P, 2, D], F32)  # [:,0]=scale  [:,1]=bias
        nc.vector.tensor_mul(SB[:, 0, :], gb[:, 0, :], inv)
        nc.vector.tensor_mul(tmp, mean2[:, 0, :], SB[:, 0, :])
        nc.vector.tensor_sub(SB[:, 1, :], gb[:, 1, :], tmp)

        # --- pass 2: Y = X * scale + bias ---
        # 7-deep tile ring: 5 stream slots + the 2 now-idle sq slots.
        # per-block plain-AP ops: broadcast (stride-0) operands drop DVE to
        # ~40% rate, so slice the tile instead
        for t in range(nt):
            if t % 7 < 5:
                xt = stream.tile([P, RB, D], F32, tag="xt")
            else:
                xt = sqpool.tile([P, RB, D], F32, tag="sq")
            nc.sync.dma_start(out=xt[:], in_=Xv[t])
            for b in range(RB):
                nc.vector.tensor_mul(xt[:, b, :], xt[:, b, :], SB[:, 0, :])
            for b in range(RB):
                nc.vector.tensor_add(xt[:, b, :], xt[:, b, :], SB[:, 1, :])
            nc.scalar.dma_start(out=Yv[t], in_=xt[:])

    nc.compile()  # bacc: register alloc, nop fusion, multi-wait event sems
    return nc


def _get_nc(np_rows=NP, n_total=N):
    key = (np_rows, n_total)
    if key not in _cache:
        _cache[key] = _build(np_rows, n_total)
    return _cache[key]


def _run(X, gamma, beta, trace=False):
    X = np.ascontiguousarray(np.asarray(X, dtype=np.float32))
    g = np.ascontiguousarray(np.asarray(gamma, dtype=np.float32).reshape(1, D))
    b = np.ascontiguousarray(np.asarray(beta, dtype=np.float32).reshape(1, D))
    rows = X.shape[0]
    per = rows // NCORES
    nc = _get_nc(per, rows)
    in_maps = [
        {"X": X[i * per:(i + 1) * per], "gamma": g, "beta": b}
        for i in range(NCORES)
    ]
    res = run_bass_kernel_spmd(nc, in_maps, list(range(NCORES)), trace=trace)
    out = np.concatenate([res.results[i]["Y"] for i in range(NCORES)], axis=0)
    return out, res


def kernel(X, gamma, beta):
    out, _ = _run(X, gamma, beta, trace=False)
    return out
